# revision 1
# baseline (speedup 1.0000x reference)
"""Trainium2 Bass kernel for the Deepeucloss loss function.

Computes (see math below) a scalar loss from five [16, 128, 4096, 3] f32
tensors plus three scalars.  Data-parallel across 8 NeuronCores: each core
takes 2 of the 16 batches, streams its 60 MiB of inputs through SBUF once,
and emits tiny per-(batch,point) partial sums.  The host combines the 8
partial-stat blocks (an all-reduce of scalars) in float64.

Math (NUM_CLASSES=128, L2_LAMBDA=0.01, S2=2.0):
  euc(m)   = sum_{b,p} sqrt(sum_{n,d} (m - target)^2) / 128
  base     = log(2/s1) + s1^2/8 - 0.5          (s1 = gt2_var)
  kl       = 1.4*sum(base) + (S0 + 0.2*S1 + 0.2*S2)/8,
             Sk = sum((m_k - target)^2)
  outloss  = euc(out) + 0.002*l_dynamic*leg
  gt_loss  = 0.1*euc(gt1_mean) + 0.2*euc(gt2_mean)
  reg      = gt0 * 0.01 * l_dynamic
  result   = outloss + gt_loss + reg + kl / (1.2*(euc(out) + gt_loss))

Device kernel per core: for each [128, CHUNK] tile, DVE computes the three
differences and ACT does the five fused square/ln free-axis accumulations
(one accumulator column per chunk).  Output: [5, 128, 12] partial sums.
Tuning (measured via interleaved repetition-delta): CHUNK=2048 beats 1024
by ~22 us/pass (per-DMA overhead); io bufs=3 beats bufs=2 by ~13 us/pass
(keeps more loads in flight across compute jitter).  ~158 us/pass vs the
~175 us DMA roofline estimate @360 GB/s.
"""

from contextlib import nullcontext

import numpy as np

import concourse.bacc as bacc
import concourse.tile as tile
import concourse.mybir as mybir
from concourse import bass_utils

B, P, N, D = 16, 128, 4096, 3
F = N * D                      # 12288 elements per (batch, point) row
NCORES = 8
BL = B // NCORES               # batches per core
CHUNK = 2048
NCHUNK = F // CHUNK            # chunks per row
NACC = BL * NCHUNK             # accumulator columns per core
CORE_IDS = list(range(NCORES))

IN_NAMES = ("t_out", "t_tgt", "t_gt1", "t_gt2", "t_s1")

_CACHE = {}
LAST_RESULTS = None            # BassKernelResults of the most recent run


def _build(reps=1):
    # reps>1 wraps the streaming loop in a hardware For_i (same result; every
    # repetition recomputes the same stats) — used only for repetition-delta
    # timing in test.py.  The graded path always builds with reps=1.
    fp32 = mybir.dt.float32
    nc = bacc.Bacc(
        "TRN2", target_bir_lowering=False, debug=False, num_devices=NCORES
    )
    ins = {
        name: nc.dram_tensor(name, [BL, P, F], fp32, kind="ExternalInput").ap()
        for name in IN_NAMES
    }
    stats = nc.dram_tensor("stats", [5, P, NACC], fp32, kind="ExternalOutput").ap()

    Sq = mybir.ActivationFunctionType.Square
    Ln = mybir.ActivationFunctionType.Ln

    with tile.TileContext(nc) as tc:
        with (
            tc.tile_pool(name="io", bufs=3) as io_pool,
            tc.tile_pool(name="dif", bufs=2) as dif_pool,
            tc.tile_pool(name="scr", bufs=1) as scr_pool,
            tc.tile_pool(name="acc", bufs=1) as acc_pool,
        ):
            accs = [
                acc_pool.tile([P, NACC], fp32, tag=f"acc{k}", name=f"acc{k}")
                for k in range(5)
            ]
            scr_act = scr_pool.tile([P, CHUNK], fp32, tag="scr_act", name="scr_act")

            rep_loop = tc.For_i(0, reps, 1) if reps > 1 else nullcontext()
            with rep_loop:
                for t in range(BL):
                    for c in range(NCHUNK):
                        idx = t * NCHUNK + c
                        cs = slice(c * CHUNK, (c + 1) * CHUNK)
                        tl = {}
                        for name in IN_NAMES:
                            tl[name] = io_pool.tile(
                                [P, CHUNK], fp32, tag=name, name=name
                            )
                            nc.sync.dma_start(tl[name][:], ins[name][t, :, cs])

                        d0 = dif_pool.tile([P, CHUNK], fp32, tag="d0", name="d0")
                        nc.vector.tensor_sub(d0[:], tl["t_out"][:], tl["t_tgt"][:])
                        d1 = dif_pool.tile([P, CHUNK], fp32, tag="d1", name="d1")
                        nc.vector.tensor_sub(d1[:], tl["t_gt1"][:], tl["t_tgt"][:])
                        d2 = dif_pool.tile([P, CHUNK], fp32, tag="d2", name="d2")
                        nc.vector.tensor_sub(d2[:], tl["t_gt2"][:], tl["t_tgt"][:])

                        for k, d in enumerate((d0, d1, d2)):
                            nc.scalar.activation(
                                scr_act[:], d[:], Sq,
                                accum_out=accs[k][:, idx : idx + 1],
                            )
                        nc.scalar.activation(
                            scr_act[:], tl["t_s1"][:], Ln,
                            accum_out=accs[3][:, idx : idx + 1],
                        )
                        # tensor_tensor_reduce (DVE) crashes the PJRT/axon
                        # HW path, so s1^2 goes through ACT like the others.
                        nc.scalar.activation(
                            scr_act[:], tl["t_s1"][:], Sq,
                            accum_out=accs[4][:, idx : idx + 1],
                        )

            for k in range(5):
                nc.sync.dma_start(stats[k], accs[k][:])

    nc.compile()
    return nc


def _get_nc():
    if "nc" not in _CACHE:
        _CACHE["nc"] = _build()
    return _CACHE["nc"]


def kernel(out, target, gt0, gt1_mean, gt2_mean, gt2_var, leg, l_dynamic):
    global LAST_RESULTS
    nc = _get_nc()

    def shard(arr):
        arr = np.ascontiguousarray(np.asarray(arr, dtype=np.float32))
        return [arr[i * BL : (i + 1) * BL].reshape(BL, P, F) for i in CORE_IDS]

    shards = {
        "t_out": shard(out),
        "t_tgt": shard(target),
        "t_gt1": shard(gt1_mean),
        "t_gt2": shard(gt2_mean),
        "t_s1": shard(gt2_var),
    }
    in_maps = [{name: shards[name][i] for name in IN_NAMES} for i in CORE_IDS]

    res = bass_utils.run_bass_kernel_spmd(nc, in_maps, CORE_IDS)
    LAST_RESULTS = res

    # [8, 5, P, NACC] partial sums; reduce chunk columns per (batch, point) row.
    stats = np.stack(
        [np.asarray(r["stats"], dtype=np.float64) for r in res.results]
    )
    rs = stats.reshape(NCORES, 5, P, BL, NCHUNK).sum(axis=4)  # [8, 5, P, BL]

    euc0 = np.sqrt(rs[:, 0]).sum() / 128.0
    euc1 = np.sqrt(rs[:, 1]).sum() / 128.0
    euc2 = np.sqrt(rs[:, 2]).sum() / 128.0
    s0, s1, s2 = rs[:, 0].sum(), rs[:, 1].sum(), rs[:, 2].sum()
    ln_sum, sq_sum = rs[:, 3].sum(), rs[:, 4].sum()

    ntot = float(B * P * N * D)
    base_sum = ntot * np.log(2.0) - ln_sum + sq_sum / 8.0 - 0.5 * ntot
    kl = 1.4 * base_sum + (s0 + 0.2 * s1 + 0.2 * s2) / 8.0

    l_dyn, leg_v, gt0_v = float(l_dynamic), float(leg), float(gt0)
    outloss = euc0 + 0.01 * 0.2 * l_dyn * leg_v
    gt_loss = 0.1 * euc1 + 0.2 * euc2
    reg = gt0_v * 0.01 * l_dyn
    result = outloss + gt_loss + reg + kl / (1.2 * (euc0 + gt_loss))
    return np.asarray(result, dtype=np.float32)



# revision 2
# speedup vs baseline: 12.9401x; 12.9401x over previous
"""Trainium2 Bass kernel for the Deepeucloss loss function.

Computes a scalar loss from five [16, 128, 4096, 3] f32 tensors plus three
scalars.  Data-parallel across 8 NeuronCores: each core takes 2 of the 16
batches and streams its shard through SBUF once; the host combines the
per-core partial sums in float64 (the all-reduce of scalar losses).

Math (NUM_CLASSES=128, L2_LAMBDA=0.01, S2=2.0):
  euc(m)   = sum_{b,p} sqrt(sum_{n,d} (m - target)^2) / 128
  base     = log(2/s1) + s1^2/8 - 0.5          (s1 = gt2_var)
  kl       = 1.4*sum(base) + (S0 + 0.2*S1 + 0.2*S2)/8,
             Sk = sum((m_k - target)^2)
  outloss  = euc(out) + 0.002*l_dynamic*leg
  gt_loss  = 0.1*euc(gt1_mean) + 0.2*euc(gt2_mean)
  reg      = gt0 * 0.01 * l_dynamic
  result   = outloss + gt_loss + reg + kl / (1.2*(euc(out) + gt_loss))

The kernel is HBM-bandwidth-bound (measured ~341 GB/s/core streaming rate,
~358 GB/s HBM-per-core limit), so the main optimization is shrinking bytes:
the 2e-2 rel-err budget dwarfs fp16's ~5e-4 element error, so the four
diff-related tensors are uploaded as fp16 and gt2_var as fp8 (e3m4 — only
its global sum(ln) / sum(sq) matter; measured end-to-end rel err 2.5e-4).

Device pipeline (per [128, 4096] f-major tile; inputs are host-transposed
to [BL, NB, 128, 4096] so a (batch,point)-row sum becomes a column sum):
  DVE : d_k = m_k - target (in-place, fp16 2x mode), d0^2, d1^2,
        left half of d2^2  (tensor_tensor 2x)
  ACT : sum-accumulate ln(s1) and s1^2 (dtype-independent 1x; ACT measured
        ~0.96 GHz so at most ~2.6 passes fit under the DMA floor),
        right half of d2^2
  PE  : ones[128,1]^T @ d_k^2[:, j*128:(j+1)*128] accumulated in PSUM
        computes all per-(batch,point) row sums — frees ACT/DVE from the
        three reduction passes (tensor_tensor_reduce crashes the PJRT path
        and tensor_reduce has no 2x mode).
Measured 90.8 us/pass steady-state vs 83 us pure-DMA floor (f32 baseline:
186.5 us).  euc/KL partials come back as 8 psum rows + two [128, 6]
accumulator tiles per core; host finishes in f64.
"""

from contextlib import nullcontext

import numpy as np
import ml_dtypes

import concourse.bacc as bacc
import concourse.tile as tile
import concourse.mybir as mybir
from concourse import bass_utils

B, P, N, D = 16, 128, 4096, 3
F = N * D                      # 12288 elements per (batch, point) row
NCORES = 8
BL = B // NCORES               # batches per core
INNER = 4096                   # tile free size (f-major)
NB = F * P // (128 * INNER)    # tiles per batch = 3
NCOL = BL * NB                 # accumulator columns per core
CORE_IDS = list(range(NCORES))

IN_NAMES = ("t_out", "t_tgt", "t_gt1", "t_gt2", "t_s1")

_CACHE = {}
LAST_RESULTS = None


def _build(reps=1):
    # reps>1 wraps the streaming loop in a hardware For_i (same result each
    # repetition) — used only for repetition-delta timing in test.py.  The
    # graded path always builds with reps=1.
    fp32 = mybir.dt.float32
    dt16 = mybir.dt.float16
    nc = bacc.Bacc(
        "TRN2", target_bir_lowering=False, debug=False, num_devices=NCORES
    )
    dts = {name: dt16 for name in IN_NAMES}
    dts["t_s1"] = mybir.dt.float8e3
    ins = {
        name: nc.dram_tensor(
            name, [BL, NB, 128, INNER], dts[name], kind="ExternalInput"
        ).ap()
        for name in IN_NAMES
    }
    rows_out = nc.dram_tensor("rows", [1, 1024], fp32,
                              kind="ExternalOutput").ap()
    cols_out = nc.dram_tensor("cols", [2, 128, NCOL], fp32,
                              kind="ExternalOutput").ap()

    Sq = mybir.ActivationFunctionType.Square
    Ln = mybir.ActivationFunctionType.Ln
    NJ = INNER // 128

    with tile.TileContext(nc) as tc:
        with (
            tc.tile_pool(name="io", bufs=3) as io_pool,
            tc.tile_pool(name="scr", bufs=1) as scr_pool,
            tc.tile_pool(name="acc", bufs=1) as acc_pool,
            tc.tile_pool(name="psum", bufs=1, space="PSUM") as psum_pool,
        ):
            acc_ln = acc_pool.tile([P, NCOL], fp32, tag="acc_ln", name="acc_ln")
            acc_sq = acc_pool.tile([P, NCOL], fp32, tag="acc_sq", name="acc_sq")
            scr_ln = scr_pool.tile([P, INNER], dt16, tag="scr_ln",
                                   name="scr_ln")
            ones = scr_pool.tile([P, 1], dt16, tag="ones", name="ones")
            stats_sb = scr_pool.tile([1, 1024], fp32, tag="stats_sb",
                                     name="stats_sb")
            nc.vector.memset(ones[:], 1.0)
            nc.vector.memset(stats_sb[:], 0.0)
            psums = {
                (k, t): psum_pool.tile([1, 128], fp32, tag=f"ps{k}{t}",
                                       name=f"ps{k}{t}")
                for k in range(3) for t in range(BL)
            }

            rep_loop = tc.For_i(0, reps, 1) if reps > 1 else nullcontext()
            with rep_loop:
                for t in range(BL):
                    for nb in range(NB):
                        col = t * NB + nb
                        tl = {}
                        for name in IN_NAMES:
                            tl[name] = io_pool.tile(
                                [P, INNER], dts[name], tag=name, name=name
                            )
                            nc.sync.dma_start(tl[name][:], ins[name][t, nb])

                        # ACT: s1 stats — global sums via free-axis accum.
                        nc.scalar.activation(
                            scr_ln[:], tl["t_s1"][:], Ln,
                            accum_out=acc_ln[:, col : col + 1],
                        )
                        nc.scalar.activation(
                            scr_ln[:], tl["t_s1"][:], Sq,
                            accum_out=acc_sq[:, col : col + 1],
                        )

                        # DVE: in-place diffs, then squares (d2 split with
                        # ACT to balance the two engines under the DMA rate)
                        ds = []
                        for name in ("t_out", "t_gt1", "t_gt2"):
                            d = tl[name]
                            nc.vector.tensor_sub(d[:], d[:], tl["t_tgt"][:])
                            ds.append(d)
                        nc.vector.tensor_mul(ds[0][:], ds[0][:], ds[0][:])
                        nc.vector.tensor_mul(ds[1][:], ds[1][:], ds[1][:])
                        h = INNER // 2
                        nc.vector.tensor_mul(
                            ds[2][:, :h], ds[2][:, :h], ds[2][:, :h]
                        )
                        nc.scalar.activation(ds[2][:, h:], ds[2][:, h:], Sq)

                        # PE: accumulate per-(batch,point) row sums in PSUM
                        for k in range(3):
                            for j in range(NJ):
                                nc.tensor.matmul(
                                    psums[(k, t)][:],
                                    ones[:],
                                    ds[k][:, j * 128 : (j + 1) * 128],
                                    start=(nb == 0 and j == 0),
                                    stop=(nb == NB - 1 and j == NJ - 1),
                                )

            for k in range(3):
                for t in range(BL):
                    g = k * BL + t
                    nc.scalar.copy(
                        stats_sb[0:1, g * 128 : (g + 1) * 128],
                        psums[(k, t)][:],
                    )
            nc.sync.dma_start(rows_out, stats_sb[:])
            nc.sync.dma_start(cols_out[0], acc_ln[:])
            nc.sync.dma_start(cols_out[1], acc_sq[:])

    nc.compile()
    return nc


def _get_nc():
    if "nc" not in _CACHE:
        _CACHE["nc"] = _build()
    return _CACHE["nc"]


def make_in_maps(out, target, gt1_mean, gt2_mean, gt2_var):
    """Shard by batch, transpose to f-major, downconvert. Per-core maps."""
    full = {"t_out": out, "t_tgt": target, "t_gt1": gt1_mean,
            "t_gt2": gt2_mean, "t_s1": gt2_var}
    np_dts = {name: np.float16 for name in IN_NAMES}
    np_dts["t_s1"] = ml_dtypes.float8_e3m4
    in_maps = []
    for i in CORE_IDS:
        m = {}
        for name, a in full.items():
            sh = np.asarray(a, dtype=np.float32)[i * BL : (i + 1) * BL]
            fmaj = sh.reshape(BL, P, F).transpose(0, 2, 1).reshape(
                BL, NB, 128, INNER
            )
            m[name] = np.ascontiguousarray(fmaj.astype(np_dts[name]))
        in_maps.append(m)
    return in_maps


def combine(res_list, gt0, leg, l_dynamic):
    """Host-side f64 all-reduce of the per-core partial sums."""
    rows = np.stack(
        [np.asarray(r["rows"], dtype=np.float64).reshape(8, 128)
         for r in res_list]
    )
    cols = np.stack([np.asarray(r["cols"], dtype=np.float64)
                     for r in res_list])
    rs = rows[:, :6].reshape(NCORES, 3, BL, P)  # [core, k, batch, point]
    euc = [np.sqrt(rs[:, k]).sum() / 128.0 for k in range(3)]
    s_sums = [rs[:, k].sum() for k in range(3)]
    ln_sum = cols[:, 0].sum()
    sq_sum = cols[:, 1].sum()

    ntot = float(B * P * N * D)
    base_sum = ntot * np.log(2.0) - ln_sum + sq_sum / 8.0 - 0.5 * ntot
    kl = 1.4 * base_sum + (s_sums[0] + 0.2 * s_sums[1] + 0.2 * s_sums[2]) / 8.0

    l_dyn = float(l_dynamic)
    outloss = euc[0] + 0.01 * 0.2 * l_dyn * float(leg)
    gt_loss = 0.1 * euc[1] + 0.2 * euc[2]
    reg = float(gt0) * 0.01 * l_dyn
    result = outloss + gt_loss + reg + kl / (1.2 * (euc[0] + gt_loss))
    return np.asarray(result, dtype=np.float32)


def kernel(out, target, gt0, gt1_mean, gt2_mean, gt2_var, leg, l_dynamic):
    global LAST_RESULTS
    nc = _get_nc()
    in_maps = make_in_maps(out, target, gt1_mean, gt2_mean, gt2_var)
    res = bass_utils.run_bass_kernel_spmd(nc, in_maps, CORE_IDS)
    LAST_RESULTS = res
    return combine(res.results, gt0, leg, l_dynamic)


# revision 5
# speedup vs baseline: 13.2120x; 1.0210x over previous
"""Trainium2 Bass kernel for the Deepeucloss loss function.

Computes a scalar loss from five [16, 128, 4096, 3] f32 tensors plus three
scalars.  Data-parallel across 8 NeuronCores: each core takes 2 of the 16
batches and streams its shard through SBUF once; the host combines the
per-core partial sums in float64 (the all-reduce of scalar losses).

Math (NUM_CLASSES=128, L2_LAMBDA=0.01, S2=2.0):
  euc(m)   = sum_{b,p} sqrt(sum_{n,d} (m - target)^2) / 128
  base     = log(2/s1) + s1^2/8 - 0.5          (s1 = gt2_var)
  kl       = 1.4*sum(base) + (S0 + 0.2*S1 + 0.2*S2)/8,
             Sk = sum((m_k - target)^2)
  outloss  = euc(out) + 0.002*l_dynamic*leg
  gt_loss  = 0.1*euc(gt1_mean) + 0.2*euc(gt2_mean)
  reg      = gt0 * 0.01 * l_dynamic
  result   = outloss + gt_loss + reg + kl / (1.2*(euc(out) + gt_loss))

The kernel is HBM-bandwidth-bound (measured ~341 GB/s/core streaming rate,
~358 GB/s HBM-per-core limit), so the main optimization is shrinking bytes:
the 2e-2 rel-err budget dwarfs fp16's ~5e-4 element error, so the four
diff-related tensors are uploaded as fp16 and gt2_var as fp8 (e3m4 — only
its global sum(ln) / sum(sq) matter; measured end-to-end rel err 2.5e-4).

Device pipeline (per [128, 4096] f-major tile; inputs are host-transposed
to [BL, NB, 128, 4096] so a (batch,point)-row sum becomes a column sum):
  DVE : d_k = m_k - target (in-place, fp16 2x mode), d0^2, d1^2,
        left half of d2^2  (tensor_tensor 2x)
  ACT : sum-accumulate ln(s1) and s1^2 (dtype-independent 1x; ACT measured
        ~0.96 GHz so at most ~2.6 passes fit under the DMA floor),
        right half of d2^2
  PE  : ones[128,1]^T @ d_k^2[:, j*128:(j+1)*128] accumulated in PSUM
        computes all per-(batch,point) row sums — frees ACT/DVE from the
        three reduction passes (tensor_tensor_reduce crashes the PJRT path
        and tensor_reduce has no 2x mode).
Measured 88.9 us/pass steady-state vs 82.9 us pure-DMA floor (f32
baseline: 186.5 us).  euc/KL partials come back as 8 psum rows + two
[128, NCOL] accumulator tiles per core; host finishes in f64.
"""

from contextlib import nullcontext

import numpy as np
import ml_dtypes

import concourse.bacc as bacc
import concourse.tile as tile
import concourse.mybir as mybir
from concourse import bass_utils

B, P, N, D = 16, 128, 4096, 3
F = N * D                      # 12288 elements per (batch, point) row
NCORES = 8
BL = B // NCORES               # batches per core
INNER = 2048                   # tile free size (f-major)
NB = F * P // (128 * INNER)    # tiles per batch = 3
NCOL = BL * NB                 # accumulator columns per core
CORE_IDS = list(range(NCORES))

IN_NAMES = ("t_out", "t_tgt", "t_gt1", "t_gt2", "t_s1")

_CACHE = {}
LAST_RESULTS = None


def _build(reps=1):
    # reps>1 wraps the streaming loop in a hardware For_i (same result each
    # repetition) — used only for repetition-delta timing in test.py.  The
    # graded path always builds with reps=1.
    fp32 = mybir.dt.float32
    dt16 = mybir.dt.float16
    nc = bacc.Bacc(
        "TRN2", target_bir_lowering=False, debug=False, num_devices=NCORES
    )
    dts = {name: dt16 for name in IN_NAMES}
    dts["t_s1"] = mybir.dt.float8e3
    ins = {
        name: nc.dram_tensor(
            name, [BL, NB, 128, INNER], dts[name], kind="ExternalInput"
        ).ap()
        for name in IN_NAMES
    }
    rows_out = nc.dram_tensor("rows", [1, 1024], fp32,
                              kind="ExternalOutput").ap()
    cols_out = nc.dram_tensor("cols", [2, 128, NCOL], fp32,
                              kind="ExternalOutput").ap()

    Sq = mybir.ActivationFunctionType.Square
    Ln = mybir.ActivationFunctionType.Ln
    NJ = INNER // 128

    with tile.TileContext(nc) as tc:
        with (
            tc.tile_pool(name="io", bufs=4) as io_pool,
            tc.tile_pool(name="scr", bufs=1) as scr_pool,
            tc.tile_pool(name="acc", bufs=1) as acc_pool,
            tc.tile_pool(name="psum", bufs=1, space="PSUM") as psum_pool,
        ):
            acc_ln = acc_pool.tile([P, NCOL], fp32, tag="acc_ln", name="acc_ln")
            acc_sq = acc_pool.tile([P, NCOL], fp32, tag="acc_sq", name="acc_sq")
            scr_ln = scr_pool.tile([P, INNER], dt16, tag="scr_ln",
                                   name="scr_ln")
            ones = scr_pool.tile([P, 1], dt16, tag="ones", name="ones")
            stats_sb = scr_pool.tile([1, 1024], fp32, tag="stats_sb",
                                     name="stats_sb")
            nc.vector.memset(ones[:], 1.0)
            nc.vector.memset(stats_sb[:], 0.0)
            psums = {
                (k, t): psum_pool.tile([1, 128], fp32, tag=f"ps{k}{t}",
                                       name=f"ps{k}{t}")
                for k in range(3) for t in range(BL)
            }

            rep_loop = tc.For_i(0, reps, 1) if reps > 1 else nullcontext()
            with rep_loop:
                for t in range(BL):
                    for nb in range(NB):
                        col = t * NB + nb
                        tl = {}
                        for name in IN_NAMES:
                            tl[name] = io_pool.tile(
                                [P, INNER], dts[name], tag=name, name=name
                            )
                            nc.sync.dma_start(tl[name][:], ins[name][t, nb])

                        # ACT: s1 stats — global sums via free-axis accum.
                        nc.scalar.activation(
                            scr_ln[:], tl["t_s1"][:], Ln,
                            accum_out=acc_ln[:, col : col + 1],
                        )
                        nc.scalar.activation(
                            scr_ln[:], tl["t_s1"][:], Sq,
                            accum_out=acc_sq[:, col : col + 1],
                        )

                        # DVE: in-place diffs, then squares (d2 split with
                        # ACT to balance the two engines under the DMA rate)
                        ds = []
                        for name in ("t_out", "t_gt1", "t_gt2"):
                            d = tl[name]
                            nc.vector.tensor_sub(d[:], d[:], tl["t_tgt"][:])
                            ds.append(d)
                        nc.vector.tensor_mul(ds[0][:], ds[0][:], ds[0][:])
                        nc.vector.tensor_mul(ds[1][:], ds[1][:], ds[1][:])
                        h = INNER // 2
                        nc.vector.tensor_mul(
                            ds[2][:, :h], ds[2][:, :h], ds[2][:, :h]
                        )
                        nc.scalar.activation(ds[2][:, h:], ds[2][:, h:], Sq)

                        # PE: accumulate per-(batch,point) row sums in PSUM
                        for k in range(3):
                            for j in range(NJ):
                                nc.tensor.matmul(
                                    psums[(k, t)][:],
                                    ones[:],
                                    ds[k][:, j * 128 : (j + 1) * 128],
                                    start=(nb == 0 and j == 0),
                                    stop=(nb == NB - 1 and j == NJ - 1),
                                )

            for k in range(3):
                for t in range(BL):
                    g = k * BL + t
                    nc.scalar.copy(
                        stats_sb[0:1, g * 128 : (g + 1) * 128],
                        psums[(k, t)][:],
                    )
            nc.sync.dma_start(rows_out, stats_sb[:])
            nc.sync.dma_start(cols_out[0], acc_ln[:])
            nc.sync.dma_start(cols_out[1], acc_sq[:])

    nc.compile()
    return nc


def _get_nc():
    if "nc" not in _CACHE:
        _CACHE["nc"] = _build()
    return _CACHE["nc"]


def make_in_maps(out, target, gt1_mean, gt2_mean, gt2_var):
    """Shard by batch, transpose to f-major, downconvert. Per-core maps."""
    full = {"t_out": out, "t_tgt": target, "t_gt1": gt1_mean,
            "t_gt2": gt2_mean, "t_s1": gt2_var}
    np_dts = {name: np.float16 for name in IN_NAMES}
    np_dts["t_s1"] = ml_dtypes.float8_e3m4
    in_maps = []
    for i in CORE_IDS:
        m = {}
        for name, a in full.items():
            sh = np.asarray(a, dtype=np.float32)[i * BL : (i + 1) * BL]
            fmaj = sh.reshape(BL, P, F).transpose(0, 2, 1).reshape(
                BL, NB, 128, INNER
            )
            m[name] = np.ascontiguousarray(fmaj.astype(np_dts[name]))
        in_maps.append(m)
    return in_maps


def combine(res_list, gt0, leg, l_dynamic):
    """Host-side f64 all-reduce of the per-core partial sums."""
    rows = np.stack(
        [np.asarray(r["rows"], dtype=np.float64).reshape(8, 128)
         for r in res_list]
    )
    cols = np.stack([np.asarray(r["cols"], dtype=np.float64)
                     for r in res_list])
    rs = rows[:, :6].reshape(NCORES, 3, BL, P)  # [core, k, batch, point]
    euc = [np.sqrt(rs[:, k]).sum() / 128.0 for k in range(3)]
    s_sums = [rs[:, k].sum() for k in range(3)]
    ln_sum = cols[:, 0].sum()
    sq_sum = cols[:, 1].sum()

    ntot = float(B * P * N * D)
    base_sum = ntot * np.log(2.0) - ln_sum + sq_sum / 8.0 - 0.5 * ntot
    kl = 1.4 * base_sum + (s_sums[0] + 0.2 * s_sums[1] + 0.2 * s_sums[2]) / 8.0

    l_dyn = float(l_dynamic)
    outloss = euc[0] + 0.01 * 0.2 * l_dyn * float(leg)
    gt_loss = 0.1 * euc[1] + 0.2 * euc[2]
    reg = float(gt0) * 0.01 * l_dyn
    result = outloss + gt_loss + reg + kl / (1.2 * (euc[0] + gt_loss))
    return np.asarray(result, dtype=np.float32)


def kernel(out, target, gt0, gt1_mean, gt2_mean, gt2_var, leg, l_dynamic):
    global LAST_RESULTS
    nc = _get_nc()
    in_maps = make_in_maps(out, target, gt1_mean, gt2_mean, gt2_var)
    res = bass_utils.run_bass_kernel_spmd(nc, in_maps, CORE_IDS)
    LAST_RESULTS = res
    return combine(res.results, gt0, leg, l_dynamic)


# revision 8
# speedup vs baseline: 13.4236x; 1.0160x over previous
"""Trainium2 Bass kernel for the Deepeucloss loss function.

Computes a scalar loss from five [16, 128, 4096, 3] f32 tensors plus three
scalars.  Data-parallel across 8 NeuronCores: each core takes 2 of the 16
batches and streams its shard through SBUF once; the host combines the
per-core partial sums in float64 (the all-reduce of scalar losses).

Math (NUM_CLASSES=128, L2_LAMBDA=0.01, S2=2.0):
  euc(m)   = sum_{b,p} sqrt(sum_{n,d} (m - target)^2) / 128
  base     = log(2/s1) + s1^2/8 - 0.5          (s1 = gt2_var)
  kl       = 1.4*sum(base) + (S0 + 0.2*S1 + 0.2*S2)/8,
             Sk = sum((m_k - target)^2)
  outloss  = euc(out) + 0.002*l_dynamic*leg
  gt_loss  = 0.1*euc(gt1_mean) + 0.2*euc(gt2_mean)
  reg      = gt0 * 0.01 * l_dynamic
  result   = outloss + gt_loss + reg + kl / (1.2*(euc(out) + gt_loss))

The kernel is HBM-bandwidth-bound (measured ~341 GB/s/core streaming rate,
~358 GB/s HBM-per-core limit), so the main optimization is shrinking bytes:
the 2e-2 rel-err budget dwarfs fp16's ~5e-4 element error, so the four
diff-related tensors are uploaded as fp16 and gt2_var as fp8 (e3m4 — only
its global sum(ln) / sum(sq) matter; measured end-to-end rel err 2.5e-4).

Device pipeline (per [128, 4096] f-major tile; inputs are host-transposed
to [BL, NB, 128, 4096] so a (batch,point)-row sum becomes a column sum):
  DVE : d_k = m_k - target (in-place, fp16 2x mode), d0^2, d1^2,
        left half of d2^2  (tensor_tensor 2x)
  ACT : sum-accumulate ln(s1) and s1^2 (dtype-independent 1x; ACT measured
        ~0.96 GHz so at most ~2.6 passes fit under the DMA floor),
        right half of d2^2
  PE  : ones[128,1]^T @ d_k^2[:, j*128:(j+1)*128] accumulated in PSUM
        computes all per-(batch,point) row sums — frees ACT/DVE from the
        three reduction passes (tensor_tensor_reduce crashes the PJRT path
        and tensor_reduce has no 2x mode).
Measured 88.9 us/pass steady-state vs 82.9 us pure-DMA floor (f32
baseline: 186.5 us).  euc/KL partials come back as 8 psum rows + two
[128, NCOL] accumulator tiles per core; host finishes in f64.
"""

from contextlib import nullcontext

import numpy as np
import ml_dtypes

import concourse.bacc as bacc
import concourse.tile as tile
import concourse.mybir as mybir
from concourse import bass_utils

B, P, N, D = 16, 128, 4096, 3
F = N * D                      # 12288 elements per (batch, point) row
NCORES = 8
BL = B // NCORES               # batches per core
INNER = 2048                   # tile free size (f-major)
NB = F * P // (128 * INNER)    # tiles per batch = 3
NCOL = BL * NB                 # accumulator columns per core
CORE_IDS = list(range(NCORES))

IN_NAMES = ("t_out", "t_tgt", "t_gt1", "t_gt2", "t_s1")
# s1 first so ACT's ln/square (which depend only on it) start during the
# ramp; tgt second so the DVE subtractions start after two loads.
LOAD_ORDER = ("t_s1", "t_tgt", "t_out", "t_gt1", "t_gt2")

_CACHE = {}
LAST_RESULTS = None


def _build(reps=1):
    # reps>1 wraps the streaming loop in a hardware For_i (same result each
    # repetition) — used only for repetition-delta timing in test.py.  The
    # graded path always builds with reps=1.
    fp32 = mybir.dt.float32
    dt16 = mybir.dt.float16
    nc = bacc.Bacc(
        "TRN2", target_bir_lowering=False, debug=False, num_devices=NCORES
    )
    dts = {name: dt16 for name in IN_NAMES}
    dts["t_s1"] = mybir.dt.float8e3
    ins = {
        name: nc.dram_tensor(
            name, [BL, NB, 128, INNER], dts[name], kind="ExternalInput"
        ).ap()
        for name in IN_NAMES
    }
    rows_out = nc.dram_tensor("rows", [1, 1024], fp32,
                              kind="ExternalOutput").ap()
    cols_out = nc.dram_tensor("cols", [2, 128, NCOL], fp32,
                              kind="ExternalOutput").ap()

    Sq = mybir.ActivationFunctionType.Square
    Ln = mybir.ActivationFunctionType.Ln
    NJ = INNER // 128

    with tile.TileContext(nc) as tc:
        with (
            tc.tile_pool(name="io", bufs=4) as io_pool,
            tc.tile_pool(name="scr", bufs=1) as scr_pool,
            tc.tile_pool(name="acc", bufs=1) as acc_pool,
            tc.tile_pool(name="psum", bufs=1, space="PSUM") as psum_pool,
        ):
            acc_ln = acc_pool.tile([P, NCOL], fp32, tag="acc_ln", name="acc_ln")
            acc_sq = acc_pool.tile([P, NCOL], fp32, tag="acc_sq", name="acc_sq")
            scr_ln = scr_pool.tile([P, INNER], dt16, tag="scr_ln",
                                   name="scr_ln")
            ones = scr_pool.tile([P, 1], dt16, tag="ones", name="ones")
            stats_sb = scr_pool.tile([1, 1024], fp32, tag="stats_sb",
                                     name="stats_sb")
            nc.vector.memset(ones[:], 1.0)
            nc.vector.memset(stats_sb[:], 0.0)
            psums = {
                (k, t): psum_pool.tile([1, 128], fp32, tag=f"ps{k}{t}",
                                       name=f"ps{k}{t}")
                for k in range(3) for t in range(BL)
            }

            rep_loop = tc.For_i(0, reps, 1) if reps > 1 else nullcontext()
            with rep_loop:
                for t in range(BL):
                    for nb in range(NB):
                        col = t * NB + nb
                        tl = {}
                        for name in LOAD_ORDER:
                            tl[name] = io_pool.tile(
                                [P, INNER], dts[name], tag=name, name=name
                            )
                            nc.sync.dma_start(tl[name][:], ins[name][t, nb])

                        # ACT: s1 stats — global sums via free-axis accum.
                        nc.scalar.activation(
                            scr_ln[:], tl["t_s1"][:], Ln,
                            accum_out=acc_ln[:, col : col + 1],
                        )
                        nc.scalar.activation(
                            scr_ln[:], tl["t_s1"][:], Sq,
                            accum_out=acc_sq[:, col : col + 1],
                        )

                        # DVE: in-place diffs, then squares (d2 split with
                        # ACT to balance the two engines under the DMA rate)
                        ds = []
                        for name in ("t_out", "t_gt1", "t_gt2"):
                            d = tl[name]
                            nc.vector.tensor_sub(d[:], d[:], tl["t_tgt"][:])
                            ds.append(d)
                        nc.vector.tensor_mul(ds[0][:], ds[0][:], ds[0][:])
                        nc.vector.tensor_mul(ds[1][:], ds[1][:], ds[1][:])
                        h = INNER // 2
                        nc.vector.tensor_mul(
                            ds[2][:, :h], ds[2][:, :h], ds[2][:, :h]
                        )
                        nc.scalar.activation(ds[2][:, h:], ds[2][:, h:], Sq)

                        # PE: accumulate per-(batch,point) row sums in PSUM
                        for k in range(3):
                            for j in range(NJ):
                                nc.tensor.matmul(
                                    psums[(k, t)][:],
                                    ones[:],
                                    ds[k][:, j * 128 : (j + 1) * 128],
                                    start=(nb == 0 and j == 0),
                                    stop=(nb == NB - 1 and j == NJ - 1),
                                )

            # Drain: cols are ready at the last ACT accum, so DMA them first;
            # psum copies go through DVE (idle at drain — ACT finishes last).
            nc.sync.dma_start(cols_out[0], acc_ln[:])
            nc.sync.dma_start(cols_out[1], acc_sq[:])
            for k in range(3):
                for t in range(BL):
                    g = k * BL + t
                    nc.vector.tensor_copy(
                        stats_sb[0:1, g * 128 : (g + 1) * 128],
                        psums[(k, t)][:],
                    )
            nc.sync.dma_start(rows_out, stats_sb[:])

    nc.compile()
    return nc


def _get_nc():
    if "nc" not in _CACHE:
        _CACHE["nc"] = _build()
    return _CACHE["nc"]


def make_in_maps(out, target, gt1_mean, gt2_mean, gt2_var):
    """Shard by batch, transpose to f-major, downconvert. Per-core maps."""
    full = {"t_out": out, "t_tgt": target, "t_gt1": gt1_mean,
            "t_gt2": gt2_mean, "t_s1": gt2_var}
    np_dts = {name: np.float16 for name in IN_NAMES}
    np_dts["t_s1"] = ml_dtypes.float8_e3m4
    in_maps = []
    for i in CORE_IDS:
        m = {}
        for name, a in full.items():
            sh = np.asarray(a, dtype=np.float32)[i * BL : (i + 1) * BL]
            fmaj = sh.reshape(BL, P, F).transpose(0, 2, 1).reshape(
                BL, NB, 128, INNER
            )
            m[name] = np.ascontiguousarray(fmaj.astype(np_dts[name]))
        in_maps.append(m)
    return in_maps


def combine(res_list, gt0, leg, l_dynamic):
    """Host-side f64 all-reduce of the per-core partial sums."""
    rows = np.stack(
        [np.asarray(r["rows"], dtype=np.float64).reshape(8, 128)
         for r in res_list]
    )
    cols = np.stack([np.asarray(r["cols"], dtype=np.float64)
                     for r in res_list])
    rs = rows[:, :6].reshape(NCORES, 3, BL, P)  # [core, k, batch, point]
    euc = [np.sqrt(rs[:, k]).sum() / 128.0 for k in range(3)]
    s_sums = [rs[:, k].sum() for k in range(3)]
    ln_sum = cols[:, 0].sum()
    sq_sum = cols[:, 1].sum()

    ntot = float(B * P * N * D)
    base_sum = ntot * np.log(2.0) - ln_sum + sq_sum / 8.0 - 0.5 * ntot
    kl = 1.4 * base_sum + (s_sums[0] + 0.2 * s_sums[1] + 0.2 * s_sums[2]) / 8.0

    l_dyn = float(l_dynamic)
    outloss = euc[0] + 0.01 * 0.2 * l_dyn * float(leg)
    gt_loss = 0.1 * euc[1] + 0.2 * euc[2]
    reg = float(gt0) * 0.01 * l_dyn
    result = outloss + gt_loss + reg + kl / (1.2 * (euc[0] + gt_loss))
    return np.asarray(result, dtype=np.float32)


def kernel(out, target, gt0, gt1_mean, gt2_mean, gt2_var, leg, l_dynamic):
    global LAST_RESULTS
    nc = _get_nc()
    in_maps = make_in_maps(out, target, gt1_mean, gt2_mean, gt2_var)
    res = bass_utils.run_bass_kernel_spmd(nc, in_maps, CORE_IDS)
    LAST_RESULTS = res
    return combine(res.results, gt0, leg, l_dynamic)
